# revision 1
# baseline (speedup 1.0000x reference)
"""Trainium2 Bass kernel for nn_CenterLossNet (center-loss softmax over classes).

Math (reference):
    f = l2_normalize(features); c = l2_normalize(centers)
    dis[n,k]  = -5 * (|f_n|^2 + |c_k|^2 - 2 f_n.c_k)        # [N, C]
    pos[n]    = dis[n, labels[n]] + bias[labels[n]]
    den[n]    = sum_k exp(dis[n,k]) - exp(dis[n,l_n]) + exp(pos[n])
    loss      = mean(log(den) - pos) + var(pos, ddof=1);  returns (loss, var)

Device does the heavy part: S = f_hat @ c_hat.T (8192x10000x512 matmul, bf16
inputs / fp32 PSUM) fused with exp(10*S + bias_n) on the scalar engine
(accum_out row-sums). Everything O(N) or O(C) runs on host in fp64.

Sharding: data-parallel over batch N across 8 cores; centers replicated.
For the row-sum the per-class |c_k|^2 term is folded as exactly 1.0 (the
normalized squared norms differ from 1 by ~1e-6, and the host applies the
mean residual correction), while pos[n] uses the exact fp32 per-label norms.
"""

import numpy as np
import ml_dtypes

import concourse.bacc as bacc
import concourse.mybir as mybir
import concourse.tile as tile
from concourse.bass_utils import run_bass_kernel_spmd

N, C, D = 8192, 10000, 512
N_CORES = 8
NS = N // N_CORES       # 1024 rows per core
P = 128                 # partitions
M_TILES = NS // P       # 8 row tiles per core
K_TILES = D // P        # 4 contraction tiles
CW = 512                # class-tile width (one PSUM bank of fp32)
C_TILES = (C + CW - 1) // CW  # 20 (19 x 512 + 272)
SCALE = 5.0
EPS = 1e-12
BF16 = ml_dtypes.bfloat16

_compiled = None
LAST_RESULTS = None


def _build():
    nc = bacc.Bacc(
        "TRN2",
        target_bir_lowering=False,
        debug=False,
        enable_asserts=False,
        num_devices=N_CORES,
    )
    ct_d = nc.dram_tensor("ct", [D, C], mybir.dt.bfloat16, kind="ExternalInput").ap()
    ft_d = nc.dram_tensor("ft", [D, NS], mybir.dt.bfloat16, kind="ExternalInput").ap()
    ab_d = nc.dram_tensor("ab", [P, M_TILES], mybir.dt.float32, kind="ExternalInput").ap()
    rs_d = nc.dram_tensor("rs", [P, M_TILES], mybir.dt.float32, kind="ExternalOutput").ap()

    with tile.TileContext(nc) as tc:
        with (
            tc.tile_pool(name="cpool", bufs=1) as cpool,
            tc.tile_pool(name="fpool", bufs=1) as fpool,
            tc.tile_pool(name="spool", bufs=1) as spool,
            tc.tile_pool(name="dpool", bufs=1) as dpool,
            tc.tile_pool(name="partpool", bufs=2) as partpool,
            tc.tile_pool(name="ppool", bufs=6, space="PSUM") as ppool,
        ):
            bias_sb = spool.tile([P, M_TILES], mybir.dt.float32, tag="bias")
            nc.sync.dma_start(out=bias_sb[:], in_=ab_d)
            acc = spool.tile([P, M_TILES], mybir.dt.float32, tag="acc")

            ft_sb = []
            for k in range(K_TILES):
                t = fpool.tile([P, NS], mybir.dt.bfloat16, tag=f"ft{k}")
                nc.sync.dma_start(out=t[:], in_=ft_d[k * P : (k + 1) * P, :])
                ft_sb.append(t)

            # centers, transposed: strip c (in compute order) x contraction k
            ct_sb = [[None] * C_TILES for _ in range(K_TILES)]
            for c in range(C_TILES):
                w = min(CW, C - c * CW)
                for k in range(K_TILES):
                    t = cpool.tile([P, CW], mybir.dt.bfloat16, tag=f"ct{k}_{c}")
                    nc.sync.dma_start(
                        out=t[:, :w],
                        in_=ct_d[k * P : (k + 1) * P, c * CW : c * CW + w],
                    )
                    ct_sb[k][c] = t

            dummy = dpool.tile([P, CW], mybir.dt.bfloat16, tag="dummy")

            for m in range(M_TILES):
                part = partpool.tile([P, C_TILES], mybir.dt.float32, tag="part")
                for c in range(C_TILES):
                    w = min(CW, C - c * CW)
                    ps = ppool.tile([P, CW], mybir.dt.float32, tag="ps")
                    for k in range(K_TILES):
                        nc.tensor.matmul(
                            ps[:, :w],
                            ft_sb[k][:, m * P : (m + 1) * P],
                            ct_sb[k][c][:, :w],
                            start=(k == 0),
                            stop=(k == K_TILES - 1),
                        )
                    nc.scalar.activation(
                        dummy[:, :w],
                        ps[:, :w],
                        mybir.ActivationFunctionType.Exp,
                        bias=bias_sb[:, m : m + 1],
                        scale=2.0 * SCALE,
                        accum_out=part[:, c : c + 1],
                    )
                nc.vector.tensor_reduce(
                    acc[:, m : m + 1],
                    part[:, 0:C_TILES],
                    axis=mybir.AxisListType.X,
                    op=mybir.AluOpType.add,
                )
            nc.sync.dma_start(out=rs_d, in_=acc[:])

    nc.compile()
    return nc


def _get_compiled():
    global _compiled
    if _compiled is None:
        _compiled = _build()
    return _compiled


def _l2n(x):
    n = np.sqrt(np.einsum("nd,nd->n", x, x, dtype=np.float32), dtype=np.float32)
    xh = x / np.maximum(n, np.float32(EPS))[:, None]
    sq = np.einsum("nd,nd->n", xh, xh, dtype=np.float32)
    return xh.astype(np.float32), sq.astype(np.float32)


def kernel(features, labels, centers, bias):
    features = np.asarray(features, dtype=np.float32)
    centers = np.asarray(centers, dtype=np.float32)
    bias = np.asarray(bias, dtype=np.float32)
    labels_i = np.asarray(labels).astype(np.int64)

    fh, f2 = _l2n(features)          # [N, D], [N]
    ch, c2 = _l2n(centers)           # [C, D], [C]

    ct16 = np.ascontiguousarray(ch.T).astype(BF16)          # [D, C]
    abias_full = (-SCALE * (f2 + np.float32(1.0))).astype(np.float32)

    in_maps = []
    for i in range(N_CORES):
        sl = slice(i * NS, (i + 1) * NS)
        ft16 = np.ascontiguousarray(fh[sl].T).astype(BF16)  # [D, NS]
        ab = np.ascontiguousarray(
            abias_full[sl].reshape(M_TILES, P).T
        )  # [P, M_TILES], n = m*128 + p
        in_maps.append({"ct": ct16, "ft": ft16, "ab": ab})

    nc = _get_compiled()
    global LAST_RESULTS
    LAST_RESULTS = run_bass_kernel_spmd(nc, in_maps, core_ids=list(range(N_CORES)))

    rowsum = np.concatenate(
        [LAST_RESULTS.results[i]["rs"].T.reshape(NS) for i in range(N_CORES)]
    ).astype(np.float64)

    # residual correction for the |c_k|^2 ~= 1 fold (mean of exp(-5*(c2-1)))
    wmean = np.exp(-SCALE * (c2.astype(np.float64) - 1.0)).mean()
    rowsum *= wmean

    # exact per-row label terms (fp32 inputs, fp64 math)
    cl = ch[labels_i]                                        # [N, D]
    dot = np.einsum("nd,nd->n", fh.astype(np.float64), cl.astype(np.float64))
    dis_l = -SCALE * (f2.astype(np.float64) + c2[labels_i].astype(np.float64) - 2.0 * dot)
    pos = dis_l + bias[labels_i, 0].astype(np.float64)

    num = np.exp(pos)
    den = rowsum - np.exp(dis_l) + num
    logits = np.log(den) - pos
    variance = np.var(pos, ddof=1)
    loss = logits.mean() + variance
    return (np.float32(loss), np.float32(variance))


# revision 4
# speedup vs baseline: 1.0371x; 1.0371x over previous
"""Trainium2 Bass kernel for nn_CenterLossNet (center-loss softmax over classes).

Math (reference):
    f = l2_normalize(features); c = l2_normalize(centers)
    dis[n,k]  = -5 * (|f_n|^2 + |c_k|^2 - 2 f_n.c_k)        # [N, C]
    pos[n]    = dis[n, labels[n]] + bias[labels[n]]
    den[n]    = sum_k exp(dis[n,k]) - exp(dis[n,l_n]) + exp(pos[n])
    loss      = mean(log(den) - pos) + var(pos, ddof=1);  returns (loss, var)

Device does the heavy part: S = f_hat @ c_hat.T (8192x10000x512 matmul, bf16
inputs / fp32 PSUM) fused with exp(10*S + bias_n) on the scalar engine
(accum_out row-sums). Everything O(N) or O(C) runs on host in fp64.

Sharding: data-parallel over batch N across 8 cores; centers replicated.
For the row-sum the per-class |c_k|^2 term is folded as exactly 1.0 (the
normalized squared norms differ from 1 by ~1e-6, and the host applies the
mean residual correction), while pos[n] uses the exact fp32 per-label norms.
"""

import numpy as np
import ml_dtypes

import concourse.bacc as bacc
import concourse.mybir as mybir
import concourse.tile as tile
from concourse.bass_utils import run_bass_kernel_spmd

N, C, D = 8192, 10000, 512
N_CORES = 8
NS = N // N_CORES       # 1024 rows per core
P = 128                 # partitions
M_TILES = NS // P       # 8 row tiles per core
K_TILES = D // P        # 4 contraction tiles
CW = 512                # matmul free-dim tile (one PSUM bank of fp32)
C_TILES = (C + CW - 1) // CW  # 20 (19 x 512 + 272)
GW = 2048               # PSUM megatile width: 4 banks, one ACTIVATE each
G_TILES = (C + GW - 1) // GW  # 5 (4 x 2048 + 1808)
SCALE = 5.0
EPS = 1e-12
BF16 = ml_dtypes.bfloat16

_compiled = None
LAST_RESULTS = None


def _build():
    nc = bacc.Bacc(
        "TRN2",
        target_bir_lowering=False,
        debug=False,
        enable_asserts=False,
        num_devices=N_CORES,
    )
    ct_d = nc.dram_tensor("ct", [D, C], mybir.dt.bfloat16, kind="ExternalInput").ap()
    ft_d = nc.dram_tensor("ft", [D, NS], mybir.dt.bfloat16, kind="ExternalInput").ap()
    ab_d = nc.dram_tensor("ab", [P, M_TILES], mybir.dt.float32, kind="ExternalInput").ap()
    rs_d = nc.dram_tensor("rs", [P, M_TILES], mybir.dt.float32, kind="ExternalOutput").ap()

    with tile.TileContext(nc) as tc:
        with (
            tc.tile_pool(name="cpool", bufs=1) as cpool,
            tc.tile_pool(name="fpool", bufs=1) as fpool,
            tc.tile_pool(name="spool", bufs=1) as spool,
            tc.tile_pool(name="dpool", bufs=1) as dpool,
            tc.tile_pool(name="partpool", bufs=2) as partpool,
            tc.tile_pool(name="ppool", bufs=2, space="PSUM") as ppool,
        ):
            bias_sb = spool.tile([P, M_TILES], mybir.dt.float32, tag="bias")
            nc.sync.dma_start(out=bias_sb[:], in_=ab_d)
            acc = spool.tile([P, M_TILES], mybir.dt.float32, tag="acc")

            ft_sb = []
            for k in range(K_TILES):
                t = fpool.tile([P, NS], mybir.dt.bfloat16, tag=f"ft{k}")
                nc.sync.dma_start(out=t[:], in_=ft_d[k * P : (k + 1) * P, :])
                ft_sb.append(t)

            # centers, transposed: strip c (in compute order) x contraction k
            ct_sb = [[None] * C_TILES for _ in range(K_TILES)]
            for c in range(C_TILES):
                w = min(CW, C - c * CW)
                for k in range(K_TILES):
                    t = cpool.tile([P, CW], mybir.dt.bfloat16, tag=f"ct{k}_{c}")
                    nc.sync.dma_start(
                        out=t[:, :w],
                        in_=ct_d[k * P : (k + 1) * P, c * CW : c * CW + w],
                    )
                    ct_sb[k][c] = t

            dummy = dpool.tile([P, GW], mybir.dt.bfloat16, tag="dummy")

            for m in range(M_TILES):
                part = partpool.tile([P, G_TILES], mybir.dt.float32, tag="part")
                for g in range(G_TILES):
                    gw = min(GW, C - g * GW)
                    n_sl = (gw + CW - 1) // CW
                    ps = ppool.tile([P, GW], mybir.dt.float32, tag="ps")
                    # k outer / slice inner: 4 consecutive matmuls share lhsT
                    for k in range(K_TILES):
                        for j in range(n_sl):
                            c = g * 4 + j
                            w = min(CW, C - c * CW)
                            nc.tensor.matmul(
                                ps[:, j * CW : j * CW + w],
                                ft_sb[k][:, m * P : (m + 1) * P],
                                ct_sb[k][c][:, :w],
                                start=(k == 0),
                                stop=(k == K_TILES - 1),
                                skip_group_check=True,
                            )
                    nc.scalar.activation(
                        dummy[:, :gw],
                        ps[:, :gw],
                        mybir.ActivationFunctionType.Exp,
                        bias=bias_sb[:, m : m + 1],
                        scale=2.0 * SCALE,
                        accum_out=part[:, g : g + 1],
                    )
                nc.vector.tensor_reduce(
                    acc[:, m : m + 1],
                    part[:, 0:G_TILES],
                    axis=mybir.AxisListType.X,
                    op=mybir.AluOpType.add,
                )
            nc.sync.dma_start(out=rs_d, in_=acc[:])

    nc.compile()
    return nc


def _get_compiled():
    global _compiled
    if _compiled is None:
        _compiled = _build()
    return _compiled


def _l2n(x):
    n = np.sqrt(np.einsum("nd,nd->n", x, x, dtype=np.float32), dtype=np.float32)
    xh = x / np.maximum(n, np.float32(EPS))[:, None]
    sq = np.einsum("nd,nd->n", xh, xh, dtype=np.float32)
    return xh.astype(np.float32), sq.astype(np.float32)


def kernel(features, labels, centers, bias):
    features = np.asarray(features, dtype=np.float32)
    centers = np.asarray(centers, dtype=np.float32)
    bias = np.asarray(bias, dtype=np.float32)
    labels_i = np.asarray(labels).astype(np.int64)

    fh, f2 = _l2n(features)          # [N, D], [N]
    ch, c2 = _l2n(centers)           # [C, D], [C]

    ct16 = np.ascontiguousarray(ch.T).astype(BF16)          # [D, C]
    abias_full = (-SCALE * (f2 + np.float32(1.0))).astype(np.float32)

    in_maps = []
    for i in range(N_CORES):
        sl = slice(i * NS, (i + 1) * NS)
        ft16 = np.ascontiguousarray(fh[sl].T).astype(BF16)  # [D, NS]
        ab = np.ascontiguousarray(
            abias_full[sl].reshape(M_TILES, P).T
        )  # [P, M_TILES], n = m*128 + p
        in_maps.append({"ct": ct16, "ft": ft16, "ab": ab})

    nc = _get_compiled()
    global LAST_RESULTS
    LAST_RESULTS = run_bass_kernel_spmd(nc, in_maps, core_ids=list(range(N_CORES)))

    rowsum = np.concatenate(
        [LAST_RESULTS.results[i]["rs"].T.reshape(NS) for i in range(N_CORES)]
    ).astype(np.float64)

    # residual correction for the |c_k|^2 ~= 1 fold (mean of exp(-5*(c2-1)))
    wmean = np.exp(-SCALE * (c2.astype(np.float64) - 1.0)).mean()
    rowsum *= wmean

    # exact per-row label terms (fp32 inputs, fp64 math)
    cl = ch[labels_i]                                        # [N, D]
    dot = np.einsum("nd,nd->n", fh.astype(np.float64), cl.astype(np.float64))
    dis_l = -SCALE * (f2.astype(np.float64) + c2[labels_i].astype(np.float64) - 2.0 * dot)
    pos = dis_l + bias[labels_i, 0].astype(np.float64)

    num = np.exp(pos)
    den = rowsum - np.exp(dis_l) + num
    logits = np.log(den) - pos
    variance = np.var(pos, ddof=1)
    loss = logits.mean() + variance
    return (np.float32(loss), np.float32(variance))


# revision 6
# speedup vs baseline: 1.2151x; 1.1717x over previous
"""Trainium2 Bass kernel for nn_CenterLossNet (center-loss softmax over classes).

Math (reference):
    f = l2_normalize(features); c = l2_normalize(centers)
    dis[n,k]  = -5 * (|f_n|^2 + |c_k|^2 - 2 f_n.c_k)        # [N, C]
    pos[n]    = dis[n, labels[n]] + bias[labels[n]]
    den[n]    = sum_k exp(dis[n,k]) - exp(dis[n,l_n]) + exp(pos[n])
    loss      = mean(log(den) - pos) + var(pos, ddof=1);  returns (loss, var)

Device does the heavy part: S = f_hat @ c_hat.T (8192x10000x512 matmul, bf16
inputs / fp32 PSUM) fused with exp(10*S + bias_n) on the scalar engine
(accum_out row-sums). Everything O(N) or O(C) runs on host in fp64.

Sharding: data-parallel over batch N across 8 cores; centers replicated.
For the row-sum the per-class |c_k|^2 term is folded as exactly 1.0 (the
normalized squared norms differ from 1 by ~1e-6, and the host applies the
mean residual correction), while pos[n] uses the exact fp32 per-label norms.
"""

import numpy as np
import ml_dtypes

import concourse.bacc as bacc
import concourse.mybir as mybir
import concourse.tile as tile
from concourse.bass_utils import run_bass_kernel_spmd

N, C, D = 8192, 10000, 512
N_CORES = 8
NS = N // N_CORES       # 1024 rows per core
P = 128                 # partitions
M_TILES = NS // P       # 8 row tiles per core
K_TILES = D // P        # 4 contraction tiles
CW = 512                # matmul free-dim tile (one PSUM bank of fp32)
C_TILES = (C + CW - 1) // CW  # 20 (19 x 512 + 272)
GW = 2048               # PSUM megatile width: 4 banks, one ACTIVATE each
G_TILES = (C + GW - 1) // GW  # 5 (4 x 2048 + 1808)
SCALE = 5.0
EPS = 1e-12
BF16 = ml_dtypes.bfloat16

_compiled = None
LAST_RESULTS = None


def _build():
    nc = bacc.Bacc(
        "TRN2",
        target_bir_lowering=False,
        debug=False,
        enable_asserts=False,
        num_devices=N_CORES,
    )
    ct_d = nc.dram_tensor("ct", [D, C], mybir.dt.bfloat16, kind="ExternalInput").ap()
    ft_d = nc.dram_tensor("ft", [D, NS], mybir.dt.bfloat16, kind="ExternalInput").ap()
    ab_d = nc.dram_tensor("ab", [P, M_TILES], mybir.dt.float32, kind="ExternalInput").ap()
    rs_d = nc.dram_tensor("rs", [P, M_TILES], mybir.dt.float32, kind="ExternalOutput").ap()

    with tile.TileContext(nc) as tc:
        with (
            tc.tile_pool(name="cpool", bufs=1) as cpool,
            tc.tile_pool(name="fpool", bufs=1) as fpool,
            tc.tile_pool(name="spool", bufs=1) as spool,
            tc.tile_pool(name="dpool", bufs=1) as dpool,
            tc.tile_pool(name="partpool", bufs=2) as partpool,
            tc.tile_pool(name="ppool", bufs=2, space="PSUM") as ppool,
        ):
            bias_sb = spool.tile([P, M_TILES], mybir.dt.float32, tag="bias")
            nc.sync.dma_start(out=bias_sb[:], in_=ab_d)
            acc = spool.tile([P, M_TILES], mybir.dt.float32, tag="acc")

            ft_sb = []
            for k in range(K_TILES):
                t = fpool.tile([P, NS], mybir.dt.bfloat16, tag=f"ft{k}")
                nc.sync.dma_start(out=t[:], in_=ft_d[k * P : (k + 1) * P, :])
                ft_sb.append(t)

            # centers, transposed: 2048-wide strips (4 KB DMA rows), strip g
            # in compute order, contraction k inner
            ct_sb = [[None] * G_TILES for _ in range(K_TILES)]
            for g in range(G_TILES):
                gw = min(GW, C - g * GW)
                for k in range(K_TILES):
                    t = cpool.tile([P, GW], mybir.dt.bfloat16, tag=f"ct{k}_{g}")
                    nc.sync.dma_start(
                        out=t[:, :gw],
                        in_=ct_d[k * P : (k + 1) * P, g * GW : g * GW + gw],
                    )
                    ct_sb[k][g] = t

            dummy = dpool.tile([P, GW], mybir.dt.bfloat16, tag="dummy")

            parts = [
                partpool.tile([P, G_TILES], mybir.dt.float32, tag=f"part{m}", name=f"part{m}")
                for m in range(M_TILES)
            ]

            # strip-outer / row-tile-inner: PE is dense as soon as strip 0 lands
            for g in range(G_TILES):
                gw = min(GW, C - g * GW)
                n_sl = (gw + CW - 1) // CW
                for m in range(M_TILES):
                    ps = ppool.tile([P, GW], mybir.dt.float32, tag="ps")
                    for k in range(K_TILES):
                        for j in range(n_sl):
                            w = min(CW, gw - j * CW)
                            nc.tensor.matmul(
                                ps[:, j * CW : j * CW + w],
                                ft_sb[k][:, m * P : (m + 1) * P],
                                ct_sb[k][g][:, j * CW : j * CW + w],
                                start=(k == 0),
                                stop=(k == K_TILES - 1),
                                skip_group_check=True,
                            )
                    nc.scalar.activation(
                        dummy[:, :gw],
                        ps[:, :gw],
                        mybir.ActivationFunctionType.Exp,
                        bias=bias_sb[:, m : m + 1],
                        scale=2.0 * SCALE,
                        accum_out=parts[m][:, g : g + 1],
                    )
            for m in range(M_TILES):
                nc.vector.tensor_reduce(
                    acc[:, m : m + 1],
                    parts[m][:, 0:G_TILES],
                    axis=mybir.AxisListType.X,
                    op=mybir.AluOpType.add,
                )
            nc.sync.dma_start(out=rs_d, in_=acc[:])

    nc.compile()
    return nc


def _get_compiled():
    global _compiled
    if _compiled is None:
        _compiled = _build()
    return _compiled


def _l2n(x):
    n = np.sqrt(np.einsum("nd,nd->n", x, x, dtype=np.float32), dtype=np.float32)
    xh = x / np.maximum(n, np.float32(EPS))[:, None]
    sq = np.einsum("nd,nd->n", xh, xh, dtype=np.float32)
    return xh.astype(np.float32), sq.astype(np.float32)


def kernel(features, labels, centers, bias):
    features = np.asarray(features, dtype=np.float32)
    centers = np.asarray(centers, dtype=np.float32)
    bias = np.asarray(bias, dtype=np.float32)
    labels_i = np.asarray(labels).astype(np.int64)

    fh, f2 = _l2n(features)          # [N, D], [N]
    ch, c2 = _l2n(centers)           # [C, D], [C]

    ct16 = np.ascontiguousarray(ch.T).astype(BF16)          # [D, C]
    abias_full = (-SCALE * (f2 + np.float32(1.0))).astype(np.float32)

    in_maps = []
    for i in range(N_CORES):
        sl = slice(i * NS, (i + 1) * NS)
        ft16 = np.ascontiguousarray(fh[sl].T).astype(BF16)  # [D, NS]
        ab = np.ascontiguousarray(
            abias_full[sl].reshape(M_TILES, P).T
        )  # [P, M_TILES], n = m*128 + p
        in_maps.append({"ct": ct16, "ft": ft16, "ab": ab})

    nc = _get_compiled()
    global LAST_RESULTS
    LAST_RESULTS = run_bass_kernel_spmd(nc, in_maps, core_ids=list(range(N_CORES)))

    rowsum = np.concatenate(
        [LAST_RESULTS.results[i]["rs"].T.reshape(NS) for i in range(N_CORES)]
    ).astype(np.float64)

    # residual correction for the |c_k|^2 ~= 1 fold (mean of exp(-5*(c2-1)))
    wmean = np.exp(-SCALE * (c2.astype(np.float64) - 1.0)).mean()
    rowsum *= wmean

    # exact per-row label terms (fp32 inputs, fp64 math)
    cl = ch[labels_i]                                        # [N, D]
    dot = np.einsum("nd,nd->n", fh.astype(np.float64), cl.astype(np.float64))
    dis_l = -SCALE * (f2.astype(np.float64) + c2[labels_i].astype(np.float64) - 2.0 * dot)
    pos = dis_l + bias[labels_i, 0].astype(np.float64)

    num = np.exp(pos)
    den = rowsum - np.exp(dis_l) + num
    logits = np.log(den) - pos
    variance = np.var(pos, ddof=1)
    loss = logits.mean() + variance
    return (np.float32(loss), np.float32(variance))


# revision 7
# speedup vs baseline: 1.8046x; 1.4851x over previous
"""Trainium2 Bass kernel for nn_CenterLossNet (center-loss softmax over classes).

Math (reference):
    f = l2_normalize(features); c = l2_normalize(centers)
    dis[n,k]  = -5 * (|f_n|^2 + |c_k|^2 - 2 f_n.c_k)        # [N, C]
    pos[n]    = dis[n, labels[n]] + bias[labels[n]]
    den[n]    = sum_k exp(dis[n,k]) - exp(dis[n,l_n]) + exp(pos[n])
    loss      = mean(log(den) - pos) + var(pos, ddof=1);  returns (loss, var)

Device does the heavy part: S = f_hat @ c_hat.T (8192x10000x512 matmul) fused
with exp(10*S + bias_n) on the scalar engine (accum_out row-sums). The matmul
runs in fp8e4m3 DoubleRow perf mode (operands pre-scaled by 2^9, two k-rows
packed per PE cell); the exp-sum averages the fp8 rounding noise down to
~1e-5 relative on the loss. Everything O(N) or O(C) runs on host in fp64,
so pos/variance use exact fp32 inputs.

Sharding: data-parallel over batch N across 8 cores; centers replicated.
For the row-sum the per-class |c_k|^2 term is folded as exactly 1.0 (the
normalized squared norms differ from 1 by ~1e-6, and the host applies the
mean residual correction), while pos[n] uses the exact fp32 per-label norms.
"""

import numpy as np
import ml_dtypes

import concourse.bacc as bacc
import concourse.mybir as mybir
import concourse.tile as tile
from concourse.bass_utils import run_bass_kernel_spmd

N, C, D = 8192, 10000, 512
N_CORES = 8
NS = N // N_CORES       # 1024 rows per core
P = 128                 # partitions
M_TILES = NS // P       # 8 row tiles per core
K2_TILES = D // (2 * P)  # 2 DoubleRow contraction tiles (256 rows each)
CW = 512                # matmul free-dim tile (one PSUM bank of fp32)
GW = 2048               # PSUM megatile width: 4 banks, one ACTIVATE each
G_TILES = (C + GW - 1) // GW  # 5 (4 x 2048 + 1808)
SCALE = 5.0
EPS = 1e-12
FP8_SCALE = 512.0       # 2^9: keeps |values| <= ~120 within e4m3 normal range
FP8 = ml_dtypes.float8_e4m3

_compiled = None
LAST_RESULTS = None


def _build():
    nc = bacc.Bacc(
        "TRN2",
        target_bir_lowering=False,
        debug=False,
        enable_asserts=False,
        num_devices=N_CORES,
    )
    ct_d = nc.dram_tensor(
        "ct", [K2_TILES, P, 2, C], mybir.dt.float8e4, kind="ExternalInput"
    ).ap()
    ft_d = nc.dram_tensor(
        "ft", [K2_TILES, P, 2, NS], mybir.dt.float8e4, kind="ExternalInput"
    ).ap()
    ab_d = nc.dram_tensor("ab", [P, M_TILES], mybir.dt.float32, kind="ExternalInput").ap()
    rs_d = nc.dram_tensor("rs", [P, M_TILES], mybir.dt.float32, kind="ExternalOutput").ap()

    with tile.TileContext(nc) as tc:
        with (
            tc.tile_pool(name="cpool", bufs=1) as cpool,
            tc.tile_pool(name="fpool", bufs=1) as fpool,
            tc.tile_pool(name="spool", bufs=1) as spool,
            tc.tile_pool(name="dpool", bufs=1) as dpool,
            tc.tile_pool(name="partpool", bufs=1) as partpool,
            tc.tile_pool(name="ppool", bufs=2, space="PSUM") as ppool,
        ):
            bias_sb = spool.tile([P, M_TILES], mybir.dt.float32, tag="bias")
            nc.sync.dma_start(out=bias_sb[:], in_=ab_d)
            acc = spool.tile([P, M_TILES], mybir.dt.float32, tag="acc")

            ft_sb = []
            for k in range(K2_TILES):
                t = fpool.tile([P, 2, NS], mybir.dt.float8e4, tag=f"ft{k}", name=f"ft{k}")
                nc.sync.dma_start(out=t[:], in_=ft_d[k])
                ft_sb.append(t)

            # centers, transposed: 2048-wide strips (2 KB DMA rows), strip g
            # in compute order, DoubleRow contraction k inner
            ct_sb = [[None] * G_TILES for _ in range(K2_TILES)]
            for g in range(G_TILES):
                gw = min(GW, C - g * GW)
                for k in range(K2_TILES):
                    t = cpool.tile(
                        [P, 2, GW], mybir.dt.float8e4, tag=f"ct{k}_{g}", name=f"ct{k}_{g}"
                    )
                    nc.sync.dma_start(
                        out=t[:, :, :gw],
                        in_=ct_d[k][:, :, g * GW : g * GW + gw],
                    )
                    ct_sb[k][g] = t

            dummy = dpool.tile([P, GW], mybir.dt.bfloat16, tag="dummy")

            parts = [
                partpool.tile([P, G_TILES], mybir.dt.float32, tag=f"part{m}", name=f"part{m}")
                for m in range(M_TILES)
            ]

            # strip-outer / row-tile-inner: PE is dense as soon as strip 0 lands
            for g in range(G_TILES):
                gw = min(GW, C - g * GW)
                n_sl = (gw + CW - 1) // CW
                for m in range(M_TILES):
                    ps = ppool.tile([P, GW], mybir.dt.float32, tag="ps")
                    for k in range(K2_TILES):
                        for j in range(n_sl):
                            w = min(CW, gw - j * CW)
                            nc.tensor.matmul(
                                ps[:, j * CW : j * CW + w],
                                ft_sb[k][:, :, m * P : (m + 1) * P],
                                ct_sb[k][g][:, :, j * CW : j * CW + w],
                                start=(k == 0),
                                stop=(k == K2_TILES - 1),
                                perf_mode=mybir.MatmulPerfMode.DoubleRow,
                                skip_group_check=True,
                            )
                    nc.scalar.activation(
                        dummy[:, :gw],
                        ps[:, :gw],
                        mybir.ActivationFunctionType.Exp,
                        bias=bias_sb[:, m : m + 1],
                        scale=2.0 * SCALE / (FP8_SCALE * FP8_SCALE),
                        accum_out=parts[m][:, g : g + 1],
                    )
            for m in range(M_TILES):
                nc.vector.tensor_reduce(
                    acc[:, m : m + 1],
                    parts[m][:, 0:G_TILES],
                    axis=mybir.AxisListType.X,
                    op=mybir.AluOpType.add,
                )
            nc.sync.dma_start(out=rs_d, in_=acc[:])

    nc.compile()
    return nc


def _get_compiled():
    global _compiled
    if _compiled is None:
        _compiled = _build()
    return _compiled


def _l2n(x):
    n = np.sqrt(np.einsum("nd,nd->n", x, x, dtype=np.float32), dtype=np.float32)
    xh = x / np.maximum(n, np.float32(EPS))[:, None]
    sq = np.einsum("nd,nd->n", xh, xh, dtype=np.float32)
    return xh.astype(np.float32), sq.astype(np.float32)


def _pack_dr(xt):
    """[D, W] fp32 (pre-scaled) -> DoubleRow fp8 [K2, P, 2, W]:
    row d = k*256 + i*128 + p  ->  out[k, p, i]."""
    d, w = xt.shape
    return np.ascontiguousarray(
        xt.reshape(K2_TILES, 2, P, w).transpose(0, 2, 1, 3)
    ).astype(FP8)


def kernel(features, labels, centers, bias):
    features = np.asarray(features, dtype=np.float32)
    centers = np.asarray(centers, dtype=np.float32)
    bias = np.asarray(bias, dtype=np.float32)
    labels_i = np.asarray(labels).astype(np.int64)

    fh, f2 = _l2n(features)          # [N, D], [N]
    ch, c2 = _l2n(centers)           # [C, D], [C]

    ct8 = _pack_dr(ch.T * np.float32(FP8_SCALE))            # [K2, P, 2, C]
    abias_full = (-SCALE * (f2 + np.float32(1.0))).astype(np.float32)

    in_maps = []
    for i in range(N_CORES):
        sl = slice(i * NS, (i + 1) * NS)
        ft8 = _pack_dr(fh[sl].T * np.float32(FP8_SCALE))    # [K2, P, 2, NS]
        ab = np.ascontiguousarray(
            abias_full[sl].reshape(M_TILES, P).T
        )  # [P, M_TILES], n = m*128 + p
        in_maps.append({"ct": ct8, "ft": ft8, "ab": ab})

    nc = _get_compiled()
    global LAST_RESULTS
    LAST_RESULTS = run_bass_kernel_spmd(nc, in_maps, core_ids=list(range(N_CORES)))

    rowsum = np.concatenate(
        [LAST_RESULTS.results[i]["rs"].T.reshape(NS) for i in range(N_CORES)]
    ).astype(np.float64)

    # residual correction for the |c_k|^2 ~= 1 fold (mean of exp(-5*(c2-1)))
    wmean = np.exp(-SCALE * (c2.astype(np.float64) - 1.0)).mean()
    rowsum *= wmean

    # exact per-row label terms (fp32 inputs, fp64 math)
    cl = ch[labels_i]                                        # [N, D]
    dot = np.einsum("nd,nd->n", fh.astype(np.float64), cl.astype(np.float64))
    dis_l = -SCALE * (f2.astype(np.float64) + c2[labels_i].astype(np.float64) - 2.0 * dot)
    pos = dis_l + bias[labels_i, 0].astype(np.float64)

    num = np.exp(pos)
    den = rowsum - np.exp(dis_l) + num
    logits = np.log(den) - pos
    variance = np.var(pos, ddof=1)
    loss = logits.mean() + variance
    return (np.float32(loss), np.float32(variance))
